# revision 16
# baseline (speedup 1.0000x reference)
"""MLA QKV projection kernel for Trainium2 (8 NeuronCores, Bass/Tile).

Computes the DeepSeek-MLA q/k/v projection:
  q  = rmsnorm(hs @ q_a_w.T) @ q_b_w.T          -> [b, H, s, 192]  (pe cols rope-interleaved)
  ckv = hs @ kv_a_w.T ; compressed, k_pe = split
  kv = rmsnorm(compressed) @ kv_b_w.T           -> k_nope, v
  out = concat([q, concat(k_nope, k_pe), pad(v)], head axis)  -> [b, 3H, s, 192]

Strategy: pure data-parallel over tokens (8192 tokens -> 1024/core); all
weights replicated. All matmuls run as float32r (tf32-class, 4x fp32 rate).
The RMSNorm layernorm weights are folded into the B projection weights on
the host; the per-token rsqrt scale is applied to the B-GEMM outputs at
PSUM-eviction time (scale commutes with the GEMM). The rope interleave
permutation is folded into the rows of q_b_w / kv_a_w on the host.
"""

import sys
import types

import numpy as np

# ---- constants (hardcoded problem shape) ----
H = 32
D_NOPE = 128
D_ROPE = 64
D_Q = 192
D_V = 128
R_KV = 512
RQ = 1536
DMODEL = 4096
EPS = 1e-6
B, S = 2, 4096
NTOK = B * S            # 8192
NCORES = 8
TPC = NTOK // NCORES    # 1024 tokens per core
MT = TPC // 128         # 8 m-tiles
KT = DMODEL // 128      # 32 k-tiles for the A GEMM

A_COLS = RQ + R_KV + D_ROPE   # 2112
A_PAD_COLS = 2304             # 4 chunks of 512 (q,q,q,ckv) + 1 of 256 (k_pe+pad)
A_CHUNKS = ((0, 512), (512, 512), (1024, 512), (1536, 512), (2048, 256))
QOUT = H * D_Q                # 6144
KVOUT = H * (D_NOPE + D_V)    # 8192
QCH = QOUT // 512             # 12 B-q chunks
KVCH = KVOUT // 512           # 16 B-kv chunks
QKT = RQ // 128               # 12
KVKT = R_KV // 128            # 4


def _ensure_env():
    for p in ("/opt/trn_rl_repo", "/root/.axon_site"):
        if p not in sys.path:
            sys.path.insert(0, p)
    # bass_utils under axon wants antenv.axon_hooks; provide a stub so
    # trace=False runs never trip on the missing module.
    if "antenv.axon_hooks" not in sys.modules:
        try:
            import antenv  # noqa: F401
            import antenv.axon_hooks  # noqa: F401
        except ImportError:
            mod = types.ModuleType("antenv.axon_hooks")
            mod._hook = None
            mod.set_axon_ntff_profile_hook = lambda h: setattr(mod, "_hook", h)
            mod.get_axon_ntff_profile_hook = lambda: mod._hook
            sys.modules["antenv.axon_hooks"] = mod
            try:
                import antenv
                antenv.axon_hooks = mod
            except ImportError:
                pass


def _perm64():
    # inverse view of x.reshape(32,2).swapaxes ->  y[k] = x[2*(k%32) + k//32]
    return np.array([2 * (k % 32) + k // 32 for k in range(64)], dtype=np.int64)


_CACHE = {}


def _build():
    if "nc" in _CACHE:
        return _CACHE["nc"]
    import os
    phases = os.environ.get("MLA_PHASES", "full")  # debug: A | AT | full
    _ensure_env()
    from concourse import bacc
    import concourse.mybir as mybir
    import concourse.tile as tile
    from concourse.masks import make_identity

    F32 = mybir.dt.float32
    F32R = mybir.dt.float32r
    AF = mybir.ActivationFunctionType
    ALU = mybir.AluOpType
    AX = mybir.AxisListType

    nc = bacc.Bacc("TRN2", target_bir_lowering=False, debug=False)
    hsT_d = nc.dram_tensor("hsT", [DMODEL, TPC], F32R, kind="ExternalInput")
    waT_d = nc.dram_tensor("waT", [DMODEL, A_PAD_COLS], F32R, kind="ExternalInput")
    qbT_d = nc.dram_tensor("qbT", [RQ, QOUT], F32R, kind="ExternalInput")
    kvbT_d = nc.dram_tensor("kvbT", [R_KV, KVOUT], F32R, kind="ExternalInput")
    out_d = nc.dram_tensor("out", [3 * H, TPC, D_Q], F32, kind="ExternalOutput")

    with tile.TileContext(nc) as tc:
        with tc.tile_pool(name="persist", bufs=1) as persist:
            a_sb = persist.tile([128, MT, RQ + R_KV], F32)      # 64KB/p token-major A out
            kpe_sb = persist.tile([128, MT, D_ROPE], F32)       # 2KB/p
            stats = persist.tile([128, MT, 4], F32)             # per-chunk sumsq
            s_q = persist.tile([128, MT], F32)
            s_kv = persist.tile([128, MT], F32)
            # ---------------- phase A: fused A GEMM ----------------
            with tc.tile_pool(name="hst", bufs=1) as hstp, \
                 tc.tile_pool(name="wa", bufs=4) as wap, \
                 tc.tile_pool(name="sqs", bufs=1) as sqsp, \
                 tc.tile_pool(name="psA", bufs=2, space="PSUM") as psA:
                # one tile per k-slice so matmuls only wait on their own DMA
                hst = [hstp.tile([128, TPC], F32R, name=f"hst{k}", tag=f"hst{k}") for k in range(KT)]
                for k in range(KT):
                    eng = nc.scalar if k % 2 == 0 else nc.gpsimd
                    eng.dma_start(out=hst[k], in_=hsT_d[k * 128:(k + 1) * 128, :])
                for g in range(2):                              # m-groups of 4
                    for c, (col0, w) in enumerate(A_CHUNKS):
                        ps = psA.tile([128, 4, 512], F32)       # 4 banks, double-buffered
                        for k in range(KT):
                            wa_t = wap.tile([128, 512], F32R, tag="wa_t")
                            nc.sync.dma_start(out=wa_t[:, :w], in_=waT_d[k * 128:(k + 1) * 128, col0:col0 + w])
                            for mi in range(4):
                                m = g * 4 + mi
                                nc.tensor.matmul(ps[:, mi, :w], hst[k][:, m * 128:(m + 1) * 128], wa_t[:, :w],
                                                 start=(k == 0), stop=(k == KT - 1))
                        for mi in range(4):
                            m = g * 4 + mi
                            if c < 4:
                                asl = a_sb[:, m, col0:col0 + w]
                                nc.scalar.activation(asl, ps[:, mi, :w], AF.Copy)
                                scr = sqsp.tile([128, 512], F32)
                                nc.scalar.activation(scr[:, :w], asl, AF.Square,
                                                     accum_out=stats[:, m, c:c + 1])
                            else:
                                nc.scalar.activation(kpe_sb[:, m, :], ps[:, mi, 0:D_ROPE], AF.Copy)

            # per-token rsqrt scales (all m at once)
            with tc.tile_pool(name="scl", bufs=1) as sclp:
                for (dst, c0, c1, dim) in ((s_q, 0, 3, RQ), (s_kv, 3, 4, R_KV)):
                    t = sclp.tile([128, MT], F32)
                    nc.vector.reduce_sum(out=t, in_=stats[:, :, c0:c1], axis=AX.X)
                    nc.vector.tensor_scalar(out=t, in0=t, scalar1=1.0 / dim, scalar2=EPS,
                                            op0=ALU.mult, op1=ALU.add)
                    nc.vector.reciprocal(t, t)
                    nc.scalar.activation(dst[:, :], t, AF.Sqrt)

            if phases == "A":
                # debug: dump a_sb + kpe + scales into out and stop
                for m in range(MT):
                    nc.sync.dma_start(out=out_d[0, m * 128:(m + 1) * 128, :],
                                      in_=a_sb[:, m, 0:D_Q])
                    nc.sync.dma_start(out=out_d[1, m * 128:(m + 1) * 128, 0:D_ROPE],
                                      in_=kpe_sb[:, m, :])
                nc.sync.dma_start(out=out_d[2, 0:128, 0:MT], in_=s_q[:, :])
                nc.sync.dma_start(out=out_d[2, 0:128, MT:2 * MT], in_=s_kv[:, :])

            # ---------------- phase T: transposes,  phase B: B GEMMs ----------------
            if phases != "A":
              with tc.tile_pool(name="at", bufs=1) as atp:
                at_q = atp.tile([128, QKT, TPC], F32R)          # 48KB/p
                at_kv = atp.tile([128, KVKT, TPC], F32R)        # 16KB/p
                ident = atp.tile([128, 128], F32)
                make_identity(nc, ident)
                with tc.tile_pool(name="psT", bufs=8, space="PSUM") as psT:
                    for m in range(MT):
                        for f in range(QKT + KVKT):
                            pt = psT.tile([128, 128], F32)
                            nc.tensor.transpose(pt, a_sb[:, m, f * 128:(f + 1) * 128], ident)
                            if f < QKT:
                                dst = at_q[:, f, m * 128:(m + 1) * 128]
                            else:
                                dst = at_kv[:, f - QKT, m * 128:(m + 1) * 128]
                            nc.vector.tensor_copy(dst, pt)

                def tok_view(h, g, d0, w):
                    # DRAM view [p, mi, w] matching ev tiles' [partition, mi, cols]
                    return out_d[h, g * 512:(g + 1) * 512, d0:d0 + w].rearrange(
                        "(mi p) w -> p mi w", p=128)

                with tc.tile_pool(name="wb", bufs=6) as wbp, \
                     tc.tile_pool(name="ev", bufs=4) as evp, \
                     tc.tile_pool(name="psB", bufs=2, space="PSUM") as psB:
                    # k_pe broadcast to all key heads (not normed, not scaled):
                    # one DMA per head covering all tokens; runs early on gpsimd
                    for h in range(H):
                        nc.gpsimd.dma_start(
                            out=out_d[H + h, :, D_NOPE:D_Q].rearrange("(mi p) w -> p mi w", p=128),
                            in_=kpe_sb[:, :, :])
                    for g in range(2):                          # m-groups of 4
                        for c in range(QCH):                    # q chunks (512 cols)
                            ps = psB.tile([128, 4, 512], F32)
                            for k in range(QKT):
                                wb_t = wbp.tile([128, 512], F32R, tag="wb_t")
                                nc.sync.dma_start(out=wb_t, in_=qbT_d[k * 128:(k + 1) * 128, c * 512:(c + 1) * 512])
                                for mi in range(4):
                                    m = g * 4 + mi
                                    nc.tensor.matmul(ps[:, mi, :], at_q[:, k, m * 128:(m + 1) * 128], wb_t,
                                                     start=(k == 0), stop=(k == QKT - 1))
                            ev = evp.tile([128, 4, 512], F32, tag="ev")
                            for mi in range(4):
                                m = g * 4 + mi
                                nc.scalar.activation(ev[:, mi, :], ps[:, mi, :], AF.Copy, scale=s_q[:, m:m + 1])
                            col = c * 512
                            end = col + 512
                            while col < end:                    # one DMA per head-piece, all 4 m-tiles
                                h = col // D_Q
                                seg_end = min(end, (h + 1) * D_Q)
                                nc.gpsimd.dma_start(
                                    out=tok_view(h, g, col - h * D_Q, seg_end - col),
                                    in_=ev[:, :, col - c * 512:seg_end - c * 512])
                                col = seg_end
                        for c in range(KVCH):                   # kv chunks (512 cols = 2 heads)
                            ps = psB.tile([128, 4, 512], F32)
                            for k in range(KVKT):
                                wb_t = wbp.tile([128, 512], F32R, tag="wb_t")
                                nc.sync.dma_start(out=wb_t, in_=kvbT_d[k * 128:(k + 1) * 128, c * 512:(c + 1) * 512])
                                for mi in range(4):
                                    m = g * 4 + mi
                                    nc.tensor.matmul(ps[:, mi, :], at_kv[:, k, m * 128:(m + 1) * 128], wb_t,
                                                     start=(k == 0), stop=(k == KVKT - 1))
                            ev = evp.tile([128, 4, 512], F32, tag="ev")
                            for mi in range(4):
                                m = g * 4 + mi
                                nc.scalar.activation(ev[:, mi, :], ps[:, mi, :], AF.Copy, scale=s_kv[:, m:m + 1])
                            for hh in range(2):
                                h = 2 * c + hh
                                nc.gpsimd.dma_start(out=tok_view(H + h, g, 0, D_NOPE),
                                                    in_=ev[:, :, hh * 256:hh * 256 + 128])
                                nc.gpsimd.dma_start(out=tok_view(2 * H + h, g, 0, D_V),
                                                    in_=ev[:, :, hh * 256 + 128:hh * 256 + 256])

    nc.compile()
    _CACHE["nc"] = nc
    return nc


def _prep_inputs(hidden_states, q_a_w, kv_a_w, q_b_w, kv_b_w, q_a_ln_w, kv_a_ln_w):
    f32 = np.float32
    hs = np.asarray(hidden_states, dtype=f32).reshape(NTOK, DMODEL)
    hsT = np.ascontiguousarray(hs.T)                      # [4096, 8192]
    perm = _perm64()

    q_a_w = np.asarray(q_a_w, dtype=f32)
    kv_a_w = np.asarray(kv_a_w, dtype=f32)
    kv_a_pe = kv_a_w[R_KV:][perm]                          # de-interleave k_pe rows
    wa = np.concatenate([q_a_w, kv_a_w[:R_KV], kv_a_pe], axis=0)   # [2112, 4096]
    waT = np.zeros((DMODEL, A_PAD_COLS), dtype=f32)
    waT[:, :A_COLS] = wa.T

    qb = np.asarray(q_b_w, dtype=f32) * np.asarray(q_a_ln_w, dtype=f32)[None, :]
    qb = qb.reshape(H, D_Q, RQ).copy()
    qb[:, D_NOPE:, :] = qb[:, D_NOPE + perm, :]            # de-interleave q_pe rows
    qbT = np.ascontiguousarray(qb.reshape(QOUT, RQ).T)     # [1536, 6144]

    kvb = np.asarray(kv_b_w, dtype=f32) * np.asarray(kv_a_ln_w, dtype=f32)[None, :]
    kvbT = np.ascontiguousarray(kvb.T)                     # [512, 8192]

    in_maps = []
    for c in range(NCORES):
        in_maps.append({
            "hsT": np.ascontiguousarray(hsT[:, c * TPC:(c + 1) * TPC]),
            "waT": waT,
            "qbT": qbT,
            "kvbT": kvbT,
        })
    return in_maps


def kernel(hidden_states, q_a_w, q_b_w, kv_a_w, kv_b_w, q_a_ln_w, kv_a_ln_w,
           _trace=False):
    _ensure_env()
    from concourse.bass_utils import run_bass_kernel_spmd

    nc = _build()
    in_maps = _prep_inputs(hidden_states, q_a_w, kv_a_w, q_b_w, kv_b_w,
                           q_a_ln_w, kv_a_ln_w)
    res = run_bass_kernel_spmd(nc, in_maps, list(range(NCORES)), trace=_trace)

    out = np.empty((B, 3 * H, S, D_Q), dtype=np.float32)
    for c in range(NCORES):
        out[c // (S // TPC), :, (c % (S // TPC)) * TPC:((c % (S // TPC)) + 1) * TPC, :] = \
            res.results[c]["out"]
    out[:, 2 * H:, :, D_V:] = 0.0      # v padding is exact zeros
    if _trace:
        kernel.last_exec_time_ns = res.exec_time_ns
        kernel.last_results = res
    return out


# revision 18
# speedup vs baseline: 1.0436x; 1.0436x over previous
"""MLA QKV projection kernel for Trainium2 (8 NeuronCores, Bass/Tile).

Computes the DeepSeek-MLA q/k/v projection:
  q  = rmsnorm(hs @ q_a_w.T) @ q_b_w.T          -> [b, H, s, 192]  (pe cols rope-interleaved)
  ckv = hs @ kv_a_w.T ; compressed, k_pe = split
  kv = rmsnorm(compressed) @ kv_b_w.T           -> k_nope, v
  out = concat([q, concat(k_nope, k_pe), pad(v)], head axis)  -> [b, 3H, s, 192]

Strategy: pure data-parallel over tokens (8192 tokens -> 1024/core); all
weights replicated. All matmuls run as float32r (tf32-class, 4x fp32 rate).
The RMSNorm layernorm weights are folded into the B projection weights on
the host; the per-token rsqrt scale is applied to the B-GEMM outputs at
PSUM-eviction time (scale commutes with the GEMM). The rope interleave
permutation is folded into the rows of q_b_w / kv_a_w on the host.
"""

import sys
import types

import numpy as np

# ---- constants (hardcoded problem shape) ----
H = 32
D_NOPE = 128
D_ROPE = 64
D_Q = 192
D_V = 128
R_KV = 512
RQ = 1536
DMODEL = 4096
EPS = 1e-6
B, S = 2, 4096
NTOK = B * S            # 8192
NCORES = 8
TPC = NTOK // NCORES    # 1024 tokens per core
MT = TPC // 128         # 8 m-tiles
KT = DMODEL // 128      # 32 k-tiles for the A GEMM

A_COLS = RQ + R_KV + D_ROPE   # 2112
A_PAD_COLS = 2304             # 4 chunks of 512 (q,q,q,ckv) + 1 of 256 (k_pe+pad)
A_CHUNKS = ((0, 512), (512, 512), (1024, 512), (1536, 512), (2048, 256))
QOUT = H * D_Q                # 6144
KVOUT = H * (D_NOPE + D_V)    # 8192
QCH = QOUT // 512             # 12 B-q chunks
KVCH = KVOUT // 512           # 16 B-kv chunks
QKT = RQ // 128               # 12
KVKT = R_KV // 128            # 4


def _ensure_env():
    for p in ("/opt/trn_rl_repo", "/root/.axon_site"):
        if p not in sys.path:
            sys.path.insert(0, p)
    # bass_utils under axon wants antenv.axon_hooks; provide a stub so
    # trace=False runs never trip on the missing module.
    if "antenv.axon_hooks" not in sys.modules:
        try:
            import antenv  # noqa: F401
            import antenv.axon_hooks  # noqa: F401
        except ImportError:
            mod = types.ModuleType("antenv.axon_hooks")
            mod._hook = None
            mod.set_axon_ntff_profile_hook = lambda h: setattr(mod, "_hook", h)
            mod.get_axon_ntff_profile_hook = lambda: mod._hook
            sys.modules["antenv.axon_hooks"] = mod
            try:
                import antenv
                antenv.axon_hooks = mod
            except ImportError:
                pass


def _perm64():
    # inverse view of x.reshape(32,2).swapaxes ->  y[k] = x[2*(k%32) + k//32]
    return np.array([2 * (k % 32) + k // 32 for k in range(64)], dtype=np.int64)


_CACHE = {}


def _build():
    if "nc" in _CACHE:
        return _CACHE["nc"]
    import os
    phases = os.environ.get("MLA_PHASES", "full")  # debug: A | AT | full
    _ensure_env()
    from concourse import bacc
    import concourse.mybir as mybir
    import concourse.tile as tile
    from concourse.masks import make_identity

    F32 = mybir.dt.float32
    F32R = mybir.dt.float32r
    AF = mybir.ActivationFunctionType
    ALU = mybir.AluOpType
    AX = mybir.AxisListType

    nc = bacc.Bacc("TRN2", target_bir_lowering=False, debug=False)
    hsT_d = nc.dram_tensor("hsT", [DMODEL, TPC], F32R, kind="ExternalInput")
    waT_d = nc.dram_tensor("waT", [DMODEL, A_PAD_COLS], F32R, kind="ExternalInput")
    qbT_d = nc.dram_tensor("qbT", [RQ, QOUT], F32R, kind="ExternalInput")
    kvbT_d = nc.dram_tensor("kvbT", [R_KV, KVOUT], F32R, kind="ExternalInput")
    out_d = nc.dram_tensor("out", [3 * H, TPC, D_Q], F32, kind="ExternalOutput")

    with tile.TileContext(nc) as tc:
        with tc.tile_pool(name="persist", bufs=1) as persist:
            a_sb = persist.tile([128, MT, RQ + R_KV], F32)      # 64KB/p token-major A out
            kpe_sb = persist.tile([128, MT, D_ROPE], F32)       # 2KB/p
            stats = persist.tile([128, MT, 4], F32)             # per-chunk sumsq
            s_q = persist.tile([128, MT], F32)
            s_kv = persist.tile([128, MT], F32)
            # ---------------- phase A: fused A GEMM ----------------
            with tc.tile_pool(name="hst", bufs=1) as hstp, \
                 tc.tile_pool(name="wa", bufs=4) as wap, \
                 tc.tile_pool(name="sqs", bufs=1) as sqsp, \
                 tc.tile_pool(name="psA", bufs=2, space="PSUM") as psA:
                # one tile per k-slice so matmuls only wait on their own DMA
                hst = [hstp.tile([128, TPC], F32R, name=f"hst{k}", tag=f"hst{k}") for k in range(KT)]
                for k in range(KT):
                    eng = nc.scalar if k % 2 == 0 else nc.gpsimd
                    eng.dma_start(out=hst[k], in_=hsT_d[k * 128:(k + 1) * 128, :])
                for g in range(2):                              # m-groups of 4
                    for c, (col0, w) in enumerate(A_CHUNKS):
                        ps = psA.tile([128, 4, 512], F32)       # 4 banks, double-buffered
                        for k in range(KT):
                            wa_t = wap.tile([128, 512], F32R, tag="wa_t")
                            nc.sync.dma_start(out=wa_t[:, :w], in_=waT_d[k * 128:(k + 1) * 128, col0:col0 + w])
                            for mi in range(4):
                                m = g * 4 + mi
                                nc.tensor.matmul(ps[:, mi, :w], hst[k][:, m * 128:(m + 1) * 128], wa_t[:, :w],
                                                 start=(k == 0), stop=(k == KT - 1))
                        for mi in range(4):
                            m = g * 4 + mi
                            if c < 4:
                                asl = a_sb[:, m, col0:col0 + w]
                                nc.scalar.activation(asl, ps[:, mi, :w], AF.Copy)
                                scr = sqsp.tile([128, 512], F32)
                                nc.scalar.activation(scr[:, :w], asl, AF.Square,
                                                     accum_out=stats[:, m, c:c + 1])
                            else:
                                nc.scalar.activation(kpe_sb[:, m, :], ps[:, mi, 0:D_ROPE], AF.Copy)

            # per-token rsqrt scales (all m at once)
            with tc.tile_pool(name="scl", bufs=1) as sclp:
                for (dst, c0, c1, dim) in ((s_q, 0, 3, RQ), (s_kv, 3, 4, R_KV)):
                    t = sclp.tile([128, MT], F32)
                    nc.vector.reduce_sum(out=t, in_=stats[:, :, c0:c1], axis=AX.X)
                    nc.vector.tensor_scalar(out=t, in0=t, scalar1=1.0 / dim, scalar2=EPS,
                                            op0=ALU.mult, op1=ALU.add)
                    nc.vector.reciprocal(t, t)
                    nc.scalar.activation(dst[:, :], t, AF.Sqrt)

            if phases == "A":
                # debug: dump a_sb + kpe + scales into out and stop
                for m in range(MT):
                    nc.sync.dma_start(out=out_d[0, m * 128:(m + 1) * 128, :],
                                      in_=a_sb[:, m, 0:D_Q])
                    nc.sync.dma_start(out=out_d[1, m * 128:(m + 1) * 128, 0:D_ROPE],
                                      in_=kpe_sb[:, m, :])
                nc.sync.dma_start(out=out_d[2, 0:128, 0:MT], in_=s_q[:, :])
                nc.sync.dma_start(out=out_d[2, 0:128, MT:2 * MT], in_=s_kv[:, :])

            # ---------------- phase T: transposes,  phase B: B GEMMs ----------------
            if phases != "A":
              with tc.tile_pool(name="at", bufs=1) as atp:
                at_q = atp.tile([128, QKT, TPC], F32R)          # 48KB/p
                at_kv = atp.tile([128, KVKT, TPC], F32R)        # 16KB/p
                ident = atp.tile([128, 128], F32)
                make_identity(nc, ident)
                def tok_view(h, g, d0, w):
                    # DRAM view [p, mi, w] matching ev tiles' [partition, mi, cols]
                    return out_d[h, g * 512:(g + 1) * 512, d0:d0 + w].rearrange(
                        "(mi p) w -> p mi w", p=128)

                with tc.tile_pool(name="wb", bufs=6) as wbp, \
                     tc.tile_pool(name="ev", bufs=4) as evp, \
                     tc.tile_pool(name="psB", bufs=2, space="PSUM") as psB:
                    # k_pe broadcast to all key heads (not normed, not scaled):
                    # one DMA per head covering all tokens; runs early on gpsimd
                    for h in range(H):
                        nc.gpsimd.dma_start(
                            out=out_d[H + h, :, D_NOPE:D_Q].rearrange("(mi p) w -> p mi w", p=128),
                            in_=kpe_sb[:, :, :])

                    def do_transposes(g):
                        # 64 transposes per m-group, 16 per psB-pool tile (4 f-tiles
                        # x 4 mi); B's first q chunk only waits on tiles f0-11.
                        for fb in range(4):
                            pt = psB.tile([128, 4, 512], F32, tag="psb", name=f"ptT{g}{fb}")
                            for fi in range(4):
                                f = fb * 4 + fi
                                for mi in range(4):
                                    m = g * 4 + mi
                                    nc.tensor.transpose(pt[:, mi, fi * 128:(fi + 1) * 128],
                                                        a_sb[:, m, f * 128:(f + 1) * 128], ident)
                                    if f < QKT:
                                        dst = at_q[:, f, m * 128:(m + 1) * 128]
                                    else:
                                        dst = at_kv[:, f - QKT, m * 128:(m + 1) * 128]
                                    nc.vector.tensor_copy(dst, pt[:, mi, fi * 128:(fi + 1) * 128])

                    do_transposes(0)
                    for g in range(2):                          # m-groups of 4
                        if g == 1:
                            do_transposes(1)
                        for c in range(QCH):                    # q chunks (512 cols)
                            ps = psB.tile([128, 4, 512], F32, tag="psb")
                            for k in range(QKT):
                                wb_t = wbp.tile([128, 512], F32R, tag="wb_t")
                                nc.sync.dma_start(out=wb_t, in_=qbT_d[k * 128:(k + 1) * 128, c * 512:(c + 1) * 512])
                                for mi in range(4):
                                    m = g * 4 + mi
                                    nc.tensor.matmul(ps[:, mi, :], at_q[:, k, m * 128:(m + 1) * 128], wb_t,
                                                     start=(k == 0), stop=(k == QKT - 1))
                            ev = evp.tile([128, 4, 512], F32, tag="ev")
                            for mi in range(4):
                                m = g * 4 + mi
                                nc.scalar.activation(ev[:, mi, :], ps[:, mi, :], AF.Copy, scale=s_q[:, m:m + 1])
                            col = c * 512
                            end = col + 512
                            while col < end:                    # one DMA per head-piece, all 4 m-tiles
                                h = col // D_Q
                                seg_end = min(end, (h + 1) * D_Q)
                                nc.gpsimd.dma_start(
                                    out=tok_view(h, g, col - h * D_Q, seg_end - col),
                                    in_=ev[:, :, col - c * 512:seg_end - c * 512])
                                col = seg_end
                        for c in range(KVCH):                   # kv chunks (512 cols = 2 heads)
                            ps = psB.tile([128, 4, 512], F32, tag="psb")
                            for k in range(KVKT):
                                wb_t = wbp.tile([128, 512], F32R, tag="wb_t")
                                nc.sync.dma_start(out=wb_t, in_=kvbT_d[k * 128:(k + 1) * 128, c * 512:(c + 1) * 512])
                                for mi in range(4):
                                    m = g * 4 + mi
                                    nc.tensor.matmul(ps[:, mi, :], at_kv[:, k, m * 128:(m + 1) * 128], wb_t,
                                                     start=(k == 0), stop=(k == KVKT - 1))
                            ev = evp.tile([128, 4, 512], F32, tag="ev")
                            for mi in range(4):
                                m = g * 4 + mi
                                nc.scalar.activation(ev[:, mi, :], ps[:, mi, :], AF.Copy, scale=s_kv[:, m:m + 1])
                            for hh in range(2):
                                h = 2 * c + hh
                                nc.gpsimd.dma_start(out=tok_view(H + h, g, 0, D_NOPE),
                                                    in_=ev[:, :, hh * 256:hh * 256 + 128])
                                nc.gpsimd.dma_start(out=tok_view(2 * H + h, g, 0, D_V),
                                                    in_=ev[:, :, hh * 256 + 128:hh * 256 + 256])

    nc.compile()
    _CACHE["nc"] = nc
    return nc


def _prep_inputs(hidden_states, q_a_w, kv_a_w, q_b_w, kv_b_w, q_a_ln_w, kv_a_ln_w):
    f32 = np.float32
    hs = np.asarray(hidden_states, dtype=f32).reshape(NTOK, DMODEL)
    hsT = np.ascontiguousarray(hs.T)                      # [4096, 8192]
    perm = _perm64()

    q_a_w = np.asarray(q_a_w, dtype=f32)
    kv_a_w = np.asarray(kv_a_w, dtype=f32)
    kv_a_pe = kv_a_w[R_KV:][perm]                          # de-interleave k_pe rows
    wa = np.concatenate([q_a_w, kv_a_w[:R_KV], kv_a_pe], axis=0)   # [2112, 4096]
    waT = np.zeros((DMODEL, A_PAD_COLS), dtype=f32)
    waT[:, :A_COLS] = wa.T

    qb = np.asarray(q_b_w, dtype=f32) * np.asarray(q_a_ln_w, dtype=f32)[None, :]
    qb = qb.reshape(H, D_Q, RQ).copy()
    qb[:, D_NOPE:, :] = qb[:, D_NOPE + perm, :]            # de-interleave q_pe rows
    qbT = np.ascontiguousarray(qb.reshape(QOUT, RQ).T)     # [1536, 6144]

    kvb = np.asarray(kv_b_w, dtype=f32) * np.asarray(kv_a_ln_w, dtype=f32)[None, :]
    kvbT = np.ascontiguousarray(kvb.T)                     # [512, 8192]

    in_maps = []
    for c in range(NCORES):
        in_maps.append({
            "hsT": np.ascontiguousarray(hsT[:, c * TPC:(c + 1) * TPC]),
            "waT": waT,
            "qbT": qbT,
            "kvbT": kvbT,
        })
    return in_maps


def kernel(hidden_states, q_a_w, q_b_w, kv_a_w, kv_b_w, q_a_ln_w, kv_a_ln_w,
           _trace=False):
    _ensure_env()
    from concourse.bass_utils import run_bass_kernel_spmd

    nc = _build()
    in_maps = _prep_inputs(hidden_states, q_a_w, kv_a_w, q_b_w, kv_b_w,
                           q_a_ln_w, kv_a_ln_w)
    res = run_bass_kernel_spmd(nc, in_maps, list(range(NCORES)), trace=_trace)

    out = np.empty((B, 3 * H, S, D_Q), dtype=np.float32)
    for c in range(NCORES):
        out[c // (S // TPC), :, (c % (S // TPC)) * TPC:((c % (S // TPC)) + 1) * TPC, :] = \
            res.results[c]["out"]
    out[:, 2 * H:, :, D_V:] = 0.0      # v padding is exact zeros
    if _trace:
        kernel.last_exec_time_ns = res.exec_time_ns
        kernel.last_results = res
    return out
